# revision 20
# baseline (speedup 1.0000x reference)
"""Multi-head self-attention (causal) Trainium2 Bass kernel.

Sharding: 8 cores = 2 batches x 4 head-groups (4 heads each).
Each core computes QKV for its heads, causal flash-style attention in a
transposed-scores layout (keys on partitions), and its partial output
projection.  Host sums the 4 per-batch partials (tensor-parallel unshard)
and adds b_proj.

Layout notes (per core):
  XT  [1024, 2048]  x[b].T  bf16        (host pre-transpose + cast)
  QT/KT per head-pair: [128, 2048] bf16 (head-dim on partitions)
  V   [2048, 4*65] bf16 per 128-token tile (tokens on partitions; 65th
                                         col = 1.0 -> PSUM row 64 = denom)
  ST  [128 keys, 1024=2x512 q] PSUM pair-blocks -> one exp -> PT bf16
  causal masking: multiplicative 0/1 bf16 masks applied to PT on GPSIMD
  OTcat [256, 2048] bf16 -> proj (lhsT) -> out [2048, 1024] f32 partial
"""

import sys
import os

for _p in ("/opt/trn_rl_repo",):
    if _p not in sys.path:
        sys.path.insert(0, _p)

import numpy as np
import ml_dtypes
from contextlib import ExitStack

import concourse.bass as bass
import concourse.tile as tile
from concourse import bacc, mybir
from concourse.bass import ts
from concourse.bass_utils import run_bass_kernel_spmd

F32 = mybir.dt.float32
BF16 = mybir.dt.bfloat16
EXP = mybir.ActivationFunctionType.Exp
BF = ml_dtypes.bfloat16

D = 1024          # d_model
S = 2048          # seq len
NT = 16           # 128-token tiles
NQS = 4           # 512-wide q supertiles
KD = 8            # 128-wide k tiles over d_model
SCALE = 1.0 / 8.0  # 1/sqrt(64)

_CACHE = {}


def build_nc():
    """Build the single-core Bass program (same NEFF runs SPMD on 8 cores)."""
    if "nc" in _CACHE:
        return _CACHE["nc"]
    nc = bacc.Bacc("TRN2", target_bir_lowering=False, debug=False)

    xt_d = nc.dram_tensor("xt", [D, S], BF16, kind="ExternalInput")
    wq_d = nc.dram_tensor("wq", [D, 256], BF16, kind="ExternalInput")
    wk_d = nc.dram_tensor("wk", [D, 256], BF16, kind="ExternalInput")
    wv_d = nc.dram_tensor("wv", [D, 256], BF16, kind="ExternalInput")
    bq_d = nc.dram_tensor("bq", [256], F32, kind="ExternalInput")
    bk_d = nc.dram_tensor("bk", [256], F32, kind="ExternalInput")
    bv_d = nc.dram_tensor("bv", [4, 65], BF16, kind="ExternalInput")
    wo_d = nc.dram_tensor("wo", [256, D], BF16, kind="ExternalInput")
    mask_d = nc.dram_tensor("mask", [256, 1024], BF16, kind="ExternalInput")
    out_d = nc.dram_tensor("out", [S, D], F32, kind="ExternalOutput")

    with tile.TileContext(nc) as tc, ExitStack() as ctx:
        xt_p = ctx.enter_context(tc.tile_pool(name="xt", bufs=8))
        w_p = ctx.enter_context(tc.tile_pool(name="w", bufs=24))
        wo_p = ctx.enter_context(tc.tile_pool(name="wo", bufs=2))
        msk_p = ctx.enter_context(tc.tile_pool(name="msk", bufs=2))
        bs_p = ctx.enter_context(tc.tile_pool(name="bs", bufs=4))
        bv_p = ctx.enter_context(tc.tile_pool(name="bvp", bufs=1))
        v_p = ctx.enter_context(tc.tile_pool(name="v", bufs=16))
        qk_p = ctx.enter_context(tc.tile_pool(name="qk", bufs=2))
        pt_p = ctx.enter_context(tc.tile_pool(name="pt", bufs=8))
        oc_p = ctx.enter_context(tc.tile_pool(name="oc", bufs=2))
        rbc_p = ctx.enter_context(tc.tile_pool(name="rbc", bufs=2))
        og_p = ctx.enter_context(tc.tile_pool(name="og", bufs=3))

        # ---- constant / weight loads ----
        xt_t = []
        for k in range(KD):
            t = xt_p.tile([128, S], BF16, tag="xt")
            eng = (nc.sync, nc.scalar, nc.gpsimd)[k % 3]
            eng.dma_start(out=t[:], in_=xt_d.ap()[ts(k, 128), :])
            xt_t.append(t)

        wq_t, wk_t, wv_t = [], [], []
        for lst, dram in ((wv_t, wv_d), (wq_t, wq_d), (wk_t, wk_d)):
            for k in range(KD):
                t = w_p.tile([128, 256], BF16, tag="w")
                nc.gpsimd.dma_start(out=t[:], in_=dram.ap()[ts(k, 128), :])
                lst.append(t)

        wo_t = []
        for ct in range(2):
            t = wo_p.tile([128, D], BF16, tag="wo")
            nc.gpsimd.dma_start(out=t[:], in_=wo_d.ap()[ts(ct, 128), :])
            wo_t.append(t)

        # multiplicative causal masks for the two diagonal kb-pair blocks
        msk_t = []
        for pi in range(2):
            t = msk_p.tile([128, 1024], BF16, tag="mk")
            nc.gpsimd.dma_start(out=t[:], in_=mask_d.ap()[ts(pi, 128), :])
            msk_t.append(t)

        bq_s, bk_s = [], []
        for hp in range(2):
            t = bs_p.tile([128, 1], F32, tag="bs")
            nc.sync.dma_start(out=t[:], in_=bq_d.ap()[ts(hp, 128), None])
            bq_s.append(t)
            t = bs_p.tile([128, 1], F32, tag="bs")
            nc.sync.dma_start(out=t[:], in_=bk_d.ap()[ts(hp, 128), None])
            bk_s.append(t)

        # per head: [bias(64) | 1.0] -- the trailing 1.0 becomes V_aug's
        # ones column (PSUM row 64 = softmax denominator after P@V_aug)
        bvb = bv_p.tile([128, 4, 65], BF16, tag="bvb")
        nc.sync.dma_start(out=bvb[:],
                          in_=bv_d.ap()[None, :, :].to_broadcast((128, 4, 65)))

        oc_t = [oc_p.tile([128, S], BF16, tag="oc", name=f"oc{i}")
                for i in range(2)]

        # ---- V = X @ Wv + bv ; QT/KT = (Wq/Wk)^T X^T + b ----
        v_t = []
        qt_t, kt_t = [], []
        with tc.tile_pool(name="v_ps", bufs=2, space="PSUM") as v_ps, \
             tc.tile_pool(name="qkv_ps", bufs=2, space="PSUM") as qkv_ps:
            for mt in range(NT):
                vt = v_p.tile([128, 4, 65], BF16, tag="v")
                nc.gpsimd.tensor_copy(vt[:, :, 64:65], bvb[:, :, 64:65])
                ps = v_ps.tile([128, 256], F32, tag="vps")
                for k in range(KD):
                    nc.tensor.matmul(ps[:], xt_t[k][:, ts(mt, 128)],
                                     wv_t[k][:],
                                     start=(k == 0), stop=(k == KD - 1))
                nc.vector.tensor_add(
                    vt[:, :, 0:64],
                    ps[:].rearrange("p (h d) -> p h d", h=4),
                    bvb[:, :, 0:64])  # psum source: stays on DVE
                v_t.append(vt)

            for hp in range(2):
                qt = qk_p.tile([128, S], BF16, tag="qt", name=f"qt{hp}")
                kt = qk_p.tile([128, S], BF16, tag="kt", name=f"kt{hp}")
                for dst, w_t, b_s in ((qt, wq_t, bq_s[hp]),
                                      (kt, wk_t, bk_s[hp])):
                    for n in range(4):
                        ps = qkv_ps.tile([128, 512], F32, tag="qkv")
                        for k in range(KD):
                            nc.tensor.matmul(
                                ps[:], w_t[k][:, ts(hp, 128)],
                                xt_t[k][:, ts(n, 512)],
                                start=(k == 0), stop=(k == KD - 1))
                        nc.scalar.add(dst[:, ts(n, 512)], ps[:], b_s[:])
                qt_t.append(qt)
                kt_t.append(kt)

        # ---- attention, two heads (rows 0:64 / 64:128) per head-pair ----
        with tc.tile_pool(name="st_ps", bufs=3, space="PSUM") as st_ps, \
             tc.tile_pool(name="ot_ps", bufs=2, space="PSUM") as ot_ps:
            for hp in range(2):
                qt, kt = qt_t[hp], kt_t[hp]
                for qs in range(NQS):
                    nkb = 4 * (qs + 1)
                    ot0 = ot_ps.tile([128, 512], F32, tag="ot")
                    ot1 = ot_ps.tile([128, 512], F32, tag="ot")
                    # rows 64:96 feed the quadrant broadcast below (only
                    # lane 0 = row 64 is used); PSUM partition ranges must
                    # be 32-aligned, and the first PV matmul (start=True)
                    # overwrites row 64 after this memset.
                    nc.vector.memset(ot0[64:96, :], 1.0)
                    nc.vector.memset(ot1[64:96, :], 1.0)
                    for kb in range(0, nkb, 2):
                        sts = []
                        for hh in (0, 1):
                            sts.append(st_ps.tile([128, 1024], F32,
                                                  tag="st", name=f"st{hh}"))
                        for half in (0, 1):
                            for hh in (0, 1):
                                r0, r1 = 64 * hh, 64 * hh + 64
                                nc.tensor.matmul(
                                    sts[hh][:, ts(half, 512)],
                                    kt[r0:r1, ts(kb + half, 128)],
                                    qt[r0:r1, ts(qs, 512)],
                                    start=True, stop=True,
                                    tile_position=(64 * hh, 0))
                        pts = []
                        for hh in (0, 1):
                            pt = pt_p.tile([128, 1024], BF16, tag="pt",
                                           name=f"pt{hh}")
                            nc.scalar.activation(pt[:], sts[hh][:], EXP,
                                                 scale=SCALE)
                            if kb >= 4 * qs:
                                pi = (kb - 4 * qs) // 2
                                nc.vector.tensor_mul(pt[:], pt[:],
                                                     msk_t[pi][:])
                            pts.append(pt)
                        for half in (0, 1):
                            for hh, ot in ((0, ot0), (1, ot1)):
                                h = 2 * hp + hh
                                nc.tensor.matmul(
                                    ot[0:65, :],
                                    v_t[kb + half][:, h, :],
                                    pts[hh][:, ts(half, 512)],
                                    start=(kb == 0 and half == 0),
                                    stop=(kb + 2 == nkb and half == 1))
                    # Softmax denominators sit on PSUM partition 64 (the
                    # ones column of V_aug).  The DVE shuffle crossbar
                    # broadcasts lane 0 of a 32-partition quadrant to all
                    # lanes; four passes fill rbc, then one fast reciprocal.
                    b0 = [0] * 32
                    ident = list(range(32))
                    rbc = rbc_p.tile([128, 512], F32, tag="rbc")
                    nc.vector.stream_shuffle(rbc[0:32, :], ot0[64:96, :], b0)
                    nc.vector.stream_shuffle(rbc[32:64, :], ot0[64:96, :], b0)
                    nc.vector.stream_shuffle(rbc[64:96, :], ot1[64:96, :], b0)
                    nc.vector.stream_shuffle(rbc[96:128, :], ot1[64:96, :], b0)
                    nc.vector.reciprocal_approx_fast(rbc[:], rbc[:])
                    # normalized O^T into the proj lhsT tile
                    nc.vector.tensor_mul(oc_t[hp][0:64, ts(qs, 512)],
                                         ot0[0:64, :], rbc[0:64, :])
                    # h1 sits at PSUM partitions 0:64 but belongs at oc rows
                    # 64:128; move quadrants with the shuffle crossbar.
                    scr = rbc_p.tile([128, 512], F32, tag="scr")
                    nc.vector.stream_shuffle(scr[64:96, :], ot1[0:32, :],
                                             ident)
                    nc.vector.stream_shuffle(scr[96:128, :], ot1[32:64, :],
                                             ident)
                    nc.vector.tensor_mul(oc_t[hp][64:128, ts(qs, 512)],
                                         scr[64:128, :], rbc[64:128, :])

        # ---- out = OTcat.T @ Wo  (partial; host adds across cores) ----
        with tc.tile_pool(name="pr_ps", bufs=4, space="PSUM") as pr_ps:
            for qt_i in range(NT):
                for n2 in range(2):
                    ps = pr_ps.tile([128, 512], F32, tag="pr")
                    for ct in range(2):
                        nc.tensor.matmul(ps[:], oc_t[ct][:, ts(qt_i, 128)],
                                         wo_t[ct][:, ts(n2, 512)],
                                         start=(ct == 0), stop=(ct == 1))
                    og = og_p.tile([128, 512], F32, tag="og")
                    nc.scalar.copy(og[:], ps[:])
                    eng = (nc.sync, nc.scalar, nc.gpsimd)[(2 * qt_i + n2) % 3]
                    eng.dma_start(out=out_d.ap()[ts(qt_i, 128), ts(n2, 512)],
                                  in_=og[:])

    nc.finalize()
    _CACHE["nc"] = nc
    return nc


def make_masks():
    """[256, 1024] bf16 multiplicative causal masks for the two diagonal
    kb-PAIR blocks of a 512-wide q supertile.  Pair pi covers key blocks
    mi = 2*pi (cols 0:512) and 2*pi+1 (cols 512:1024):
    mask[128*pi + kk, 512*half + qq] = 1 iff key 128*(2*pi+half)+kk <= qq."""
    kk = np.arange(128)[:, None]
    qq = np.arange(512)[None, :]
    rows = []
    for pi in range(2):
        halves = [(128 * (2 * pi + half) + kk <= qq) for half in range(2)]
        rows.append(np.concatenate(halves, axis=1))
    return np.concatenate(rows, axis=0).astype(BF)


def make_in_maps(x, w_qkv, b_qkv, w_proj):
    x = np.asarray(x, dtype=np.float32)
    w_qkv = np.asarray(w_qkv, dtype=np.float32)
    b_qkv = np.asarray(b_qkv, dtype=np.float32)
    w_proj = np.asarray(w_proj, dtype=np.float32)
    mask = make_masks()
    in_maps = []
    for c in range(8):
        b, g = divmod(c, 4)
        cs = slice(256 * g, 256 * g + 256)
        in_maps.append({
            "xt": np.ascontiguousarray(x[b].T).astype(BF),
            "wq": np.ascontiguousarray(w_qkv[:, 0 * D:][:, cs]).astype(BF),
            "wk": np.ascontiguousarray(w_qkv[:, 1 * D:][:, cs]).astype(BF),
            "wv": np.ascontiguousarray(w_qkv[:, 2 * D:][:, cs]).astype(BF),
            "bq": np.ascontiguousarray(b_qkv[0 * D:][cs]),
            "bk": np.ascontiguousarray(b_qkv[1 * D:][cs]),
            "bv": np.concatenate(
                [b_qkv[2 * D:][cs].reshape(4, 64),
                 np.ones((4, 1), np.float32)], axis=1).astype(BF),
            "wo": np.ascontiguousarray(w_proj[cs, :]).astype(BF),
            "mask": mask,
        })
    return in_maps


def combine(partials, b_proj):
    b_proj = np.asarray(b_proj, dtype=np.float32)
    out0 = partials[0] + partials[1] + partials[2] + partials[3]
    out1 = partials[4] + partials[5] + partials[6] + partials[7]
    return (np.stack([out0, out1]) + b_proj[None, None, :]).astype(np.float32)


def run(x, w_qkv, b_qkv, w_proj, b_proj, trace=False):
    nc = build_nc()
    in_maps = make_in_maps(x, w_qkv, b_qkv, w_proj)
    res = run_bass_kernel_spmd(nc, in_maps, core_ids=list(range(8)),
                               trace=trace)
    partials = [r["out"] for r in res.results]
    return combine(partials, b_proj), res


def kernel(x, w_qkv, b_qkv, w_proj, b_proj):
    out, _ = run(x, w_qkv, b_qkv, w_proj, b_proj)
    return out
